# revision 22
# baseline (speedup 1.0000x reference)
"""Trainium2 Bass kernel for AttentionReadoutAtom (global-softmax segment reduce).

Math:  scores = x @ w + b ; attn = softmax(scores over all N) ;
       out[s] = sum_{i: label_i = s} attn_i * x_i          -> [50000, 128]

Softmax is shift/scale invariant, so exp(score) without max-subtraction is
safe (scores ~ N(0,1)) and the bias b cancels.  Using xw = x * w:

    out[s, d] = sum_{i in s} e_i * xw_i[d] / (w[d] * Z),   Z = sum_i e_i

Sharding (host, inside kernel()): sort rows by segment label, greedily pack
whole segments into blocks of 1024 rows (8 row-tiles of 128) covering <=128
distinct segments; blocks are dealt contiguously to 8 cores.  Every segment
lives in exactly one block, so no cross-core combine is needed; the only
global quantity is Z, reduced on the host (the hint's denominator
all-reduce).

Device, per block of 8 row-tiles (Tile framework schedules all engines):
  * score'[p, t]: one 4x-mode DVE tensor_scalar with accum_out per tile
    (immediate scalars keep the fast mode; fp16 out), or one grouped 1x
    tensor_reduce per chunk (ATTN_SCORE).  Column 128 of each tile is 1.0
    and col 129 is 0.0, so score' = score + 1: a constant softmax shift
    that cancels in the normalization.
  * me[p, s] = e'_p * onehot(lab_p)[s], engine-split per ATTN_TILE_PATTERN
    (one char per row-tile):
      s: ScalarE fused ACTIVATE me = Exp(logmask_fp8 + score'): the
         host-shipped fp8 tile is 0 at (p, lab_p), -96 elsewhere; score'
         rides the per-partition bias port.  exp(-96+s) == 0.
      g: ONE GpSimd local_scatter per block builds ALL g tiles: e' values
         (bf16, from a batched exp) scatter into a zeroed [128, G*128]
         tile at host-shipped int16 indices tile*128 + lab (pads: -1,
         ignored; real tiles form a prefix per partition so negatives
         always trail).
      v: DVE dual-ALU tensor_scalar me = (iota == lab) * e'.
  * psum[s, w] += me^T @ xw_aug_tile  (TensorE; two blocks share one PSUM
    bank).  Column 128 of the result is the per-segment sum of e' (pad
    rows have me == 0 everywhere) -> Z on host.
  * evict 2-block psum -> chunk SBUF tile (DVE or ScalarE copy,
    ATTN_EVICT) -> one DMA per 4-block chunk.

DMA issue cost on the SP engine is ~625 ns per DMA, so inputs ship as ONE
uint8 tensor per 4-block chunk (xw_aug bytes + fp8 mask bytes per block,
bitcast on SBUF into bf16 / fp8 views).

Host epilogue: Z = sum of column-128, scatter block rows to the full
output, divide by w[d] * Z.
"""

import os
import numpy as np
import ml_dtypes

# ---------------------------------------------------------------- constants
N = 500000
D = 128
NUM_SEGMENTS = 50000
N_CORES = 8
P = 128
TPB = 8                   # row tiles per block
ROWS_PER_BLOCK = TPB * P  # 1024
MAX_SEGS_PER_BLOCK = 128
W = 130                   # cols per tile in xw_aug: 128 xw + 1.0 + 0.0
CB = int(os.environ.get("ATTN_CB", "4"))  # blocks per chunk (one input+output DMA)

# per-row-tile engine assignment: s=ScalarE fused, g=GpSimd scatter, v=DVE
PATTERN = os.environ.get("ATTN_TILE_PATTERN", "sssggggg")
assert len(PATTERN) == TPB and set(PATTERN) <= set("sgv")
NS = PATTERN.count("s")
NG = PATTERN.count("g")
NV = PATTERN.count("v")
IDW = ((NG + 1 + 3) // 4) * 4 if NG else 0  # num_idxs, 16B-aligned slices
EGW = 8                   # e' slots per block (16B-aligned data slices)
SCORE = os.environ.get("ATTN_SCORE", "reduce")  # "ts4x" | "reduce"
EVICT = os.environ.get("ATTN_EVICT", "dve")   # "dve" | "act"
RG = int(os.environ.get("ATTN_RG", "4"))      # blocks per score reduce
BPB = TPB * W * 2 + NS * P  # bytes per block in the combined input tensor

_COMPILED = {}


# ---------------------------------------------------------------- device code
def _build_kernel(B):
    import concourse.bacc as bacc
    import concourse.mybir as mybir
    from concourse.tile import TileContext
    from concourse import library_config

    f32 = mybir.dt.float32
    bf16 = mybir.dt.bfloat16
    f8 = mybir.dt.float8e4
    i16 = mybir.dt.int16
    f16 = mybir.dt.float16
    u8 = mybir.dt.uint8
    Alu = mybir.AluOpType
    Act = mybir.ActivationFunctionType
    Ax = mybir.AxisListType

    s_tiles = [t for t, c in enumerate(PATTERN) if c == "s"]
    g_tiles = [t for t, c in enumerate(PATTERN) if c == "g"]
    v_tiles = [t for t, c in enumerate(PATTERN) if c == "v"]
    if g_tiles:
        assert g_tiles == list(range(g_tiles[0], g_tiles[0] + NG)), \
            "g tiles must be contiguous for the grouped scatter"

    NCHUNK = (B + CB - 1) // CB

    nc = bacc.Bacc("TRN2", target_bir_lowering=False, debug=False,
                   num_devices=N_CORES)

    xm_d = nc.dram_tensor("xm", [NCHUNK, P, CB * BPB], u8,
                          kind="ExternalInput")
    labi_d = nc.dram_tensor("labi", [P, max(1, B * IDW)], i16,
                            kind="ExternalInput")
    labf_d = nc.dram_tensor("labf", [P, max(1, B * NV)], f32,
                            kind="ExternalInput")
    out_d = nc.dram_tensor("out", [NCHUNK, P, CB * W], f32,
                           kind="ExternalOutput")
    dbg_d = None
    if os.environ.get("ATTN_DEBUG_MEG"):
        dbg_d = nc.dram_tensor("dbg", [B, P, NG * P], mybir.dt.bfloat16,
                               kind="ExternalOutput")

    with TileContext(nc) as tc:
        with tc.tile_pool(name="const", bufs=1) as cpool, \
             tc.tile_pool(name="xmp", bufs=4) as xmp, \
             tc.tile_pool(name="scp", bufs=4) as scp, \
             tc.tile_pool(name="mep", bufs=8) as mep, \
             tc.tile_pool(name="mgp", bufs=6) as mgp, \
             tc.tile_pool(name="evp", bufs=3) as evp, \
             tc.tile_pool(name="psum", bufs=6, space="PSUM") as psp:

            iota_b = None
            if NV:
                iota_i = cpool.tile([P, P], mybir.dt.int32)
                nc.gpsimd.iota(iota_i[:], pattern=[[1, P]], base=0,
                               channel_multiplier=0)
                iota_b = cpool.tile([P, P], bf16)
                nc.vector.tensor_copy(iota_b[:], iota_i[:])
            if NG:
                nc.gpsimd.load_library(library_config.local_scatter)

            labi = cpool.tile([P, max(1, B * IDW)], i16)
            nc.sync.dma_start(labi[:], labi_d.ap()[:, :])
            labf = cpool.tile([P, max(1, B * NV)], f32)
            nc.sync.dma_start(labf[:], labf_d.ap()[:, :])

            for ch in range(NCHUNK):
                blocks = list(range(ch * CB, min((ch + 1) * CB, B)))
                nb = len(blocks)

                xm_t = xmp.tile([P, CB * BPB], u8, tag="xm")
                nc.sync.dma_start(xm_t[:, :nb * BPB],
                                  xm_d.ap()[ch, :, :nb * BPB])

                sc_t = scp.tile([P, nb * TPB],
                                f32 if SCORE == "ttr" else f16, tag="sc")
                eg_t = scp.tile([P, max(1, nb * EGW) + IDW], bf16, tag="eg")
                ev_t = scp.tile([P, max(1, nb * NV)], f32, tag="ev")
                junk = scp.tile([P, W], bf16, tag="junk")

                xw_views, mk_views = [], []
                for bi in range(nb):
                    blk = xm_t[:, bi * BPB:(bi + 1) * BPB]
                    xw_views.append(blk[:, :TPB * W * 2].bitcast(bf16))
                    mk_views.append(blk[:, TPB * W * 2:].bitcast(f8))

                if SCORE == "ttr":
                    for bi in range(nb):
                        for t in range(TPB):
                            half = xw_views[bi][:, t * W:(t + 1) * W]
                            with nc.allow_low_precision(
                                    reason="fp32 internal accum"):
                                nc.vector.tensor_tensor_reduce(
                                    out=junk[:, :W // 2],
                                    in0=half[:, 0:W // 2],
                                    in1=half[:, W // 2:W],
                                    scale=1.0, scalar=0.0,
                                    op0=Alu.add, op1=Alu.add,
                                    accum_out=sc_t[:, bi * TPB + t:
                                                   bi * TPB + t + 1])
                elif SCORE == "ts4x":
                    for bi in range(nb):
                        for t in range(TPB):
                            with nc.allow_low_precision(
                                    reason="fp32 internal accum"):
                                nc.vector.tensor_scalar(
                                    out=junk[:],
                                    in0=xw_views[bi][:, t * W:(t + 1) * W],
                                    scalar1=1.0, scalar2=0.0,
                                    op0=Alu.mult, op1=Alu.add,
                                    accum_out=sc_t[:, bi * TPB + t:
                                                   bi * TPB + t + 1])
                else:
                    xw_all = (xm_t[:, :nb * BPB].bitcast(bf16)
                              .rearrange("p (b z) -> p b z", z=BPB // 2)
                              [:, :, :TPB * W]
                              .rearrange("p b (t w) -> p b t w", w=W))
                    for r0 in range(0, nb, RG):
                        r1 = min(r0 + RG, nb)
                        with nc.allow_low_precision(
                                reason="fp32 internal accum"):
                            nc.vector.tensor_reduce(
                                out=sc_t[:, r0 * TPB:r1 * TPB],
                                in_=xw_all[:, r0:r1], axis=Ax.X, op=Alu.add)

                sc3 = sc_t[:].rearrange("p (b t) -> p b t", t=TPB)
                for tl, et, nk, kw in ((g_tiles, eg_t, NG, EGW),
                                       (v_tiles, ev_t, NV, NV)):
                    if not nk:
                        continue
                    e3 = et[:, :nb * kw].rearrange("p (b k) -> p b k",
                                                   k=kw)[:, :, :nk]
                    r0 = 0
                    for j in range(1, nk + 1):
                        if j == nk or tl[j] != tl[j - 1] + 1:
                            nc.scalar.activation(
                                out=e3[:, :, r0:j],
                                in_=sc3[:, :, tl[r0]:tl[r0] + (j - r0)],
                                func=Act.Exp)
                            r0 = j

                ev = evp.tile([P, CB * W], f32, tag="evc")
                for bi, b in enumerate(blocks):
                    xw_v, mk_v = xw_views[bi], mk_views[bi]
                    if bi % 2 == 0:
                        # full PSUM bank: both 130-col halves stay inside
                        # one bank (matmul accumulation cannot cross banks)
                        ps2 = psp.tile([P, 512], f32, tag="acc")
                    ps = ps2[:, (bi % 2) * W:(bi % 2) * W + W]
                    me_g = None
                    if NG:
                        me_g = mgp.tile([P, NG * P], bf16, tag="meg")
                        nc.gpsimd.local_scatter(
                            me_g[:], eg_t[:, bi * EGW:bi * EGW + IDW],
                            labi[:, b * IDW:(b + 1) * IDW],
                            channels=P, num_elems=NG * P, num_idxs=IDW)
                    if dbg_d is not None and NG:
                        nc.sync.dma_start(dbg_d.ap()[b, :, :], me_g[:])
                    si = 0
                    for t in range(TPB):
                        if PATTERN[t] == "s":
                            me = mep.tile([P, P], bf16, tag="me")
                            nc.scalar.activation(
                                out=me[:], in_=mk_v[:, si * P:(si + 1) * P],
                                func=Act.Exp,
                                bias=sc_t[:, bi * TPB + t:bi * TPB + t + 1],
                                scale=1.0)
                            si += 1
                            lhs = me[:]
                        elif PATTERN[t] == "g":
                            j = g_tiles.index(t)
                            lhs = me_g[:, j * P:(j + 1) * P]
                        else:
                            me = mep.tile([P, P], bf16, tag="me")
                            lcol = b * NV + v_tiles.index(t)
                            ecol = bi * NV + v_tiles.index(t)
                            nc.vector.tensor_scalar(
                                out=me[:], in0=iota_b[:],
                                scalar1=labf[:, lcol:lcol + 1],
                                scalar2=ev_t[:, ecol:ecol + 1],
                                op0=Alu.is_equal, op1=Alu.mult)
                            lhs = me[:]
                        nc.tensor.matmul(ps, lhsT=lhs,
                                         rhs=xw_v[:, t * W:(t + 1) * W],
                                         start=(t == 0), stop=(t == TPB - 1))
                    if bi % 2 == 1 or bi == nb - 1:
                        npsb = (bi % 2) + 1
                        lo = (bi - npsb + 1) * W
                        if EVICT == "dve":
                            nc.vector.tensor_copy(
                                ev[:, lo:lo + npsb * W],
                                ps2[:, :npsb * W])
                        else:
                            nc.scalar.copy(ev[:, lo:lo + npsb * W],
                                           ps2[:, :npsb * W])
                nc.sync.dma_start(out_d.ap()[ch, :, :nb * W],
                                  ev[:, :nb * W])

    nc.compile()
    return nc


# ---------------------------------------------------------------- host side
def _pack_blocks(counts):
    blocks = []
    s, nseg = 0, len(counts)
    while s < nseg:
        rows, s0 = 0, s
        while s < nseg and s - s0 < MAX_SEGS_PER_BLOCK:
            c = counts[s]
            if rows + c > ROWS_PER_BLOCK:
                break
            rows += int(c)
            s += 1
        assert s > s0, f"segment {s0} with {counts[s0]} rows exceeds a block"
        blocks.append((s0, s, rows))
    return blocks


def _numpy_fallback(x, labels, w, b):
    scores = x.astype(np.float64) @ w.astype(np.float64) + float(b)
    scores -= scores.max()
    e = np.exp(scores)
    a = e / e.sum()
    out = np.zeros((NUM_SEGMENTS, x.shape[1]), np.float64)
    np.add.at(out, labels, x * a[:, None])
    return out.astype(np.float32)


def kernel(x, monomer_labels_i, attn_w, attn_b):
    from concourse import bass_utils

    x = np.ascontiguousarray(np.asarray(x, dtype=np.float32))
    labels = np.asarray(monomer_labels_i).astype(np.int64)
    w = np.asarray(attn_w, dtype=np.float32)
    b = np.float32(np.asarray(attn_b))

    counts = np.bincount(labels, minlength=NUM_SEGMENTS)
    if np.abs(w).min() < 1e-30 or counts.max() > ROWS_PER_BLOCK:
        return _numpy_fallback(x, labels, w, b)

    order = np.argsort(labels, kind="stable")
    labels_s = labels[order]
    blocks = _pack_blocks(counts)
    nblocks = len(blocks)
    B = (nblocks + N_CORES - 1) // N_CORES
    NCHUNK = (B + CB - 1) // CB

    s0_arr = np.fromiter((blk[0] for blk in blocks), np.int64, nblocks)
    s1_arr = np.fromiter((blk[1] for blk in blocks), np.int64, nblocks)
    rows_arr = np.fromiter((blk[2] for blk in blocks), np.int64, nblocks)
    r0_arr = np.zeros(nblocks, np.int64)
    np.cumsum(rows_arr[:-1], out=r0_arr[1:])

    # per-row placement (all rows, sorted order)
    blk_r = np.repeat(np.arange(nblocks), rows_arr)
    pos = np.arange(N) - r0_arr[blk_r]
    tile_r = (pos >> 7).astype(np.int64)
    p_r = (pos & 127).astype(np.int64)
    lab_rel = (labels_s - s0_arr[blk_r]).astype(np.int64)
    core_r = blk_r // B
    bil_r = blk_r - core_r * B

    s_tiles = [t for t, c in enumerate(PATTERN) if c == "s"]
    g_tiles = [t for t, c in enumerate(PATTERN) if c == "g"]
    v_tiles = [t for t, c in enumerate(PATTERN) if c == "v"]
    s_slot = np.full(TPB, -1, np.int64)
    g_slot = np.full(TPB, -1, np.int64)
    v_slot = np.full(TPB, -1, np.int64)
    for j, t in enumerate(s_tiles):
        s_slot[t] = j
    for j, t in enumerate(g_tiles):
        g_slot[t] = j
    for j, t in enumerate(v_tiles):
        v_slot[t] = j

    xw = (x[order] * w[None, :]).astype(ml_dtypes.bfloat16)

    # combined xw_aug + mask bytes, [cores, B, P, BPB] u8
    xw_blk = np.zeros((N_CORES, B, P, TPB, W), ml_dtypes.bfloat16)
    xw_blk[:, :, :, :, 128] = 1.0
    xw_blk[core_r, bil_r, p_r, tile_r, 0:128] = xw

    byte_neg = np.array(-96.0, ml_dtypes.float8_e4m3).view(np.uint8)
    mk_blk = np.full((N_CORES, B, P, NS * P), byte_neg, np.uint8)
    m = s_slot[tile_r] >= 0
    mk_blk[core_r[m], bil_r[m], p_r[m],
           s_slot[tile_r[m]] * P + lab_rel[m]] = \
        np.array(0.0, ml_dtypes.float8_e4m3).view(np.uint8)

    xm_all = np.concatenate(
        [xw_blk.reshape(N_CORES, B, P, TPB * W).view(np.uint8)
         .reshape(N_CORES, B, P, TPB * W * 2),
         mk_blk], axis=3)                       # [cores, B, P, BPB]
    pad_blocks = NCHUNK * CB - B
    if pad_blocks:
        xm_all = np.concatenate(
            [xm_all, np.zeros((N_CORES, pad_blocks, P, BPB), np.uint8)],
            axis=1)
    xm_all = (xm_all.reshape(N_CORES, NCHUNK, CB, P, BPB)
              .transpose(0, 1, 3, 2, 4)
              .reshape(N_CORES, NCHUNK, P, CB * BPB))
    xm_all = np.ascontiguousarray(xm_all)

    labi_all = np.full((N_CORES, P, max(1, B * IDW)), -1, np.int16)
    if NG:
        m = g_slot[tile_r] >= 0
        labi_all[core_r[m], p_r[m],
                 bil_r[m] * IDW + g_slot[tile_r[m]]] = \
            (g_slot[tile_r[m]] * P + lab_rel[m]).astype(np.int16)
    labf_all = np.full((N_CORES, P, max(1, B * NV)), 500.0, np.float32)
    if NV:
        m = v_slot[tile_r] >= 0
        labf_all[core_r[m], p_r[m],
                 bil_r[m] * NV + v_slot[tile_r[m]]] = lab_rel[m]

    in_maps = [{"xm": xm_all[c],
                "labi": labi_all[c],
                "labf": labf_all[c]} for c in range(N_CORES)]

    key = (B, PATTERN, SCORE, EVICT, RG, CB)
    if key not in _COMPILED:
        _COMPILED[key] = _build_kernel(B)
    nc = _COMPILED[key]

    res = bass_utils.run_bass_kernel_spmd(nc, in_maps,
                                          core_ids=list(range(N_CORES)))

    # ---- gather / unshard
    Z = 0.0
    out = np.zeros((NUM_SEGMENTS, D), np.float32)
    for c in range(N_CORES):
        od = res.results[c]["out"].reshape(NCHUNK, P, CB, W)
        Z += float(od[:, :, :, 128].astype(np.float64).sum())
        od = od.transpose(0, 2, 1, 3)           # [NCHUNK, CB, P, W]
        for bi in range(B):
            gi = c * B + bi
            if gi >= nblocks:
                continue
            s0, s1 = int(s0_arr[gi]), int(s1_arr[gi])
            out[s0:s1] = od[bi // CB, bi % CB, :s1 - s0, :128]
    out /= (w[None, :].astype(np.float64) * Z)
    return out.astype(np.float32)


if __name__ == "__main__":
    from ref_io import get
    inputs, expected = get()
    out = kernel(**inputs)
    err = np.abs(out - expected)
    print("absmax err:", err.max(), "scale-rel:",
          err.max() / np.abs(expected).max())


# revision 23
# speedup vs baseline: 1.1442x; 1.1442x over previous
"""Trainium2 Bass kernel for AttentionReadoutAtom (global-softmax segment reduce).

Math:  scores = x @ w + b ; attn = softmax(scores over all N) ;
       out[s] = sum_{i: label_i = s} attn_i * x_i          -> [50000, 128]

Softmax is shift/scale invariant, so exp(score) without max-subtraction is
safe (scores ~ N(0,1)) and the bias b cancels.  Using xw = x * w:

    out[s, d] = sum_{i in s} e_i * xw_i[d] / (w[d] * Z),   Z = sum_i e_i

Sharding (host, inside kernel()): sort rows by segment label, greedily pack
whole segments into blocks of 1024 rows (8 row-tiles of 128) covering <=128
distinct segments; blocks are dealt contiguously to 8 cores.  Every segment
lives in exactly one block, so no cross-core combine is needed; the only
global quantity is Z, reduced on the host (the hint's denominator
all-reduce).

Device, per block of 8 row-tiles (Tile framework schedules all engines):
  * score'[p, t]: one 4x-mode DVE tensor_scalar with accum_out per tile
    (immediate scalars keep the fast mode; fp16 out), or one grouped 1x
    tensor_reduce per chunk (ATTN_SCORE).  Column 128 of each tile is 1.0
    and col 129 is 0.0, so score' = score + 1: a constant softmax shift
    that cancels in the normalization.
  * me[p, s] = e'_p * onehot(lab_p)[s], engine-split per ATTN_TILE_PATTERN
    (one char per row-tile):
      s: ScalarE fused ACTIVATE me = Exp(logmask_fp8 + score'): the
         host-shipped fp8 tile is 0 at (p, lab_p), -96 elsewhere; score'
         rides the per-partition bias port.  exp(-96+s) == 0.
      g: ONE GpSimd local_scatter per block builds ALL g tiles: e' values
         (bf16, from a batched exp) scatter into a zeroed [128, G*128]
         tile at host-shipped int16 indices tile*128 + lab (pads: -1,
         ignored; real tiles form a prefix per partition so negatives
         always trail).
      v: DVE dual-ALU tensor_scalar me = (iota == lab) * e'.
  * psum[s, w] += me^T @ xw_aug_tile  (TensorE; two blocks share one PSUM
    bank).  Column 128 of the result is the per-segment sum of e' (pad
    rows have me == 0 everywhere) -> Z on host.
  * evict 2-block psum -> chunk SBUF tile (DVE or ScalarE copy,
    ATTN_EVICT) -> one DMA per 4-block chunk.

DMA issue cost on the SP engine is ~625 ns per DMA, so inputs ship as ONE
uint8 tensor per 4-block chunk (xw_aug bytes + fp8 mask bytes per block,
bitcast on SBUF into bf16 / fp8 views).

Host epilogue: Z = sum of column-128, scatter block rows to the full
output, divide by w[d] * Z.
"""

import os
import numpy as np
import ml_dtypes

# ---------------------------------------------------------------- constants
N = 500000
D = 128
NUM_SEGMENTS = 50000
N_CORES = 8
P = 128
TPB = 8                   # row tiles per block
ROWS_PER_BLOCK = TPB * P  # 1024
MAX_SEGS_PER_BLOCK = 128
W = 130                   # cols per tile in xw_aug: 128 xw + 1.0 + 0.0
CB = int(os.environ.get("ATTN_CB", "4"))  # blocks per chunk (one input+output DMA)

# per-row-tile engine assignment: s=ScalarE fused, g=GpSimd scatter, v=DVE
PATTERN = os.environ.get("ATTN_TILE_PATTERN", "sssggggg")
assert len(PATTERN) == TPB and set(PATTERN) <= set("sgv")
NS = PATTERN.count("s")
NG = PATTERN.count("g")
NV = PATTERN.count("v")
IDW = ((NG + 1 + 3) // 4) * 4 if NG else 0  # num_idxs, 16B-aligned slices
EGW = 8                   # e' slots per block (16B-aligned data slices)
SCORE = os.environ.get("ATTN_SCORE", "reduce")  # "ts4x" | "reduce"
EVICT = os.environ.get("ATTN_EVICT", "dve")   # "dve" | "act"
RG = int(os.environ.get("ATTN_RG", "4"))      # blocks per score reduce
SG = int(os.environ.get("ATTN_SG", "2"))      # blocks per grouped scatter
BPB = TPB * W * 2 + NS * P  # bytes per block in the combined input tensor

_COMPILED = {}


# ---------------------------------------------------------------- device code
def _build_kernel(B):
    import concourse.bacc as bacc
    import concourse.mybir as mybir
    from concourse.tile import TileContext
    from concourse import library_config

    f32 = mybir.dt.float32
    bf16 = mybir.dt.bfloat16
    f8 = mybir.dt.float8e4
    i16 = mybir.dt.int16
    f16 = mybir.dt.float16
    u8 = mybir.dt.uint8
    Alu = mybir.AluOpType
    Act = mybir.ActivationFunctionType
    Ax = mybir.AxisListType

    s_tiles = [t for t, c in enumerate(PATTERN) if c == "s"]
    g_tiles = [t for t, c in enumerate(PATTERN) if c == "g"]
    v_tiles = [t for t, c in enumerate(PATTERN) if c == "v"]
    if g_tiles:
        assert g_tiles == list(range(g_tiles[0], g_tiles[0] + NG)), \
            "g tiles must be contiguous for the grouped scatter"

    NCHUNK = (B + CB - 1) // CB

    nc = bacc.Bacc("TRN2", target_bir_lowering=False, debug=False,
                   num_devices=N_CORES)

    xm_d = nc.dram_tensor("xm", [NCHUNK, P, CB * BPB], u8,
                          kind="ExternalInput")
    labi_d = nc.dram_tensor("labi", [P, max(1, B * IDW)], i16,
                            kind="ExternalInput")
    labf_d = nc.dram_tensor("labf", [P, max(1, B * NV)], f32,
                            kind="ExternalInput")
    out_d = nc.dram_tensor("out", [NCHUNK, P, CB * W], f32,
                           kind="ExternalOutput")
    dbg_d = None
    if os.environ.get("ATTN_DEBUG_MEG"):
        dbg_d = nc.dram_tensor("dbg", [B, P, NG * P], mybir.dt.bfloat16,
                               kind="ExternalOutput")

    with TileContext(nc) as tc:
        with tc.tile_pool(name="const", bufs=1) as cpool, \
             tc.tile_pool(name="xmp", bufs=4) as xmp, \
             tc.tile_pool(name="scp", bufs=4) as scp, \
             tc.tile_pool(name="mep", bufs=8) as mep, \
             tc.tile_pool(name="mgp", bufs=6) as mgp, \
             tc.tile_pool(name="evp", bufs=3) as evp, \
             tc.tile_pool(name="psum", bufs=6, space="PSUM") as psp:

            iota_b = None
            if NV:
                iota_i = cpool.tile([P, P], mybir.dt.int32)
                nc.gpsimd.iota(iota_i[:], pattern=[[1, P]], base=0,
                               channel_multiplier=0)
                iota_b = cpool.tile([P, P], bf16)
                nc.vector.tensor_copy(iota_b[:], iota_i[:])
            if NG:
                nc.gpsimd.load_library(library_config.local_scatter)

            labi = cpool.tile([P, max(1, B * IDW)], i16)
            nc.sync.dma_start(labi[:], labi_d.ap()[:, :])
            labf = cpool.tile([P, max(1, B * NV)], f32)
            nc.sync.dma_start(labf[:], labf_d.ap()[:, :])

            for ch in range(NCHUNK):
                blocks = list(range(ch * CB, min((ch + 1) * CB, B)))
                nb = len(blocks)

                xm_t = xmp.tile([P, CB * BPB], u8, tag="xm")
                nc.sync.dma_start(xm_t[:, :nb * BPB],
                                  xm_d.ap()[ch, :, :nb * BPB])

                sc_t = scp.tile([P, nb * TPB],
                                f32 if SCORE == "ttr" else f16, tag="sc")
                eg_t = scp.tile([P, max(1, nb * EGW) + IDW], bf16, tag="eg")
                ev_t = scp.tile([P, max(1, nb * NV)], f32, tag="ev")
                junk = scp.tile([P, W], bf16, tag="junk")

                xw_views, mk_views = [], []
                for bi in range(nb):
                    blk = xm_t[:, bi * BPB:(bi + 1) * BPB]
                    xw_views.append(blk[:, :TPB * W * 2].bitcast(bf16))
                    mk_views.append(blk[:, TPB * W * 2:].bitcast(f8))

                if SCORE == "ttr":
                    for bi in range(nb):
                        for t in range(TPB):
                            half = xw_views[bi][:, t * W:(t + 1) * W]
                            with nc.allow_low_precision(
                                    reason="fp32 internal accum"):
                                nc.vector.tensor_tensor_reduce(
                                    out=junk[:, :W // 2],
                                    in0=half[:, 0:W // 2],
                                    in1=half[:, W // 2:W],
                                    scale=1.0, scalar=0.0,
                                    op0=Alu.add, op1=Alu.add,
                                    accum_out=sc_t[:, bi * TPB + t:
                                                   bi * TPB + t + 1])
                elif SCORE == "ts4x":
                    for bi in range(nb):
                        for t in range(TPB):
                            with nc.allow_low_precision(
                                    reason="fp32 internal accum"):
                                nc.vector.tensor_scalar(
                                    out=junk[:],
                                    in0=xw_views[bi][:, t * W:(t + 1) * W],
                                    scalar1=1.0, scalar2=0.0,
                                    op0=Alu.mult, op1=Alu.add,
                                    accum_out=sc_t[:, bi * TPB + t:
                                                   bi * TPB + t + 1])
                else:
                    xw_all = (xm_t[:, :nb * BPB].bitcast(bf16)
                              .rearrange("p (b z) -> p b z", z=BPB // 2)
                              [:, :, :TPB * W]
                              .rearrange("p b (t w) -> p b t w", w=W))
                    for r0 in range(0, nb, RG):
                        r1 = min(r0 + RG, nb)
                        with nc.allow_low_precision(
                                reason="fp32 internal accum"):
                            nc.vector.tensor_reduce(
                                out=sc_t[:, r0 * TPB:r1 * TPB],
                                in_=xw_all[:, r0:r1], axis=Ax.X, op=Alu.add)

                sc3 = sc_t[:].rearrange("p (b t) -> p b t", t=TPB)
                for tl, et, nk, kw in ((g_tiles, eg_t, NG, EGW),
                                       (v_tiles, ev_t, NV, NV)):
                    if not nk:
                        continue
                    e3 = et[:, :nb * kw].rearrange("p (b k) -> p b k",
                                                   k=kw)[:, :, :nk]
                    r0 = 0
                    for j in range(1, nk + 1):
                        if j == nk or tl[j] != tl[j - 1] + 1:
                            nc.scalar.activation(
                                out=e3[:, :, r0:j],
                                in_=sc3[:, :, tl[r0]:tl[r0] + (j - r0)],
                                func=Act.Exp)
                            r0 = j

                ev = evp.tile([P, CB * W], f32, tag="evc")
                for bi, b in enumerate(blocks):
                    xw_v, mk_v = xw_views[bi], mk_views[bi]
                    if bi % 2 == 0:
                        # full PSUM bank: both 130-col halves stay inside
                        # one bank (matmul accumulation cannot cross banks)
                        ps2 = psp.tile([P, 512], f32, tag="acc")
                    ps = ps2[:, (bi % 2) * W:(bi % 2) * W + W]
                    if NG and bi % SG == 0:
                        sgn = min(SG, nb - bi)
                        me_g = mgp.tile([P, sgn * NG * P], bf16, tag="meg")
                        nc.gpsimd.local_scatter(
                            me_g[:],
                            eg_t[:, bi * EGW:bi * EGW + sgn * IDW],
                            labi[:, b * IDW:(b + sgn) * IDW],
                            channels=P, num_elems=sgn * NG * P,
                            num_idxs=sgn * IDW)
                    if dbg_d is not None and NG:
                        nc.sync.dma_start(dbg_d.ap()[b, :, :], me_g[:])
                    si = 0
                    for t in range(TPB):
                        if PATTERN[t] == "s":
                            me = mep.tile([P, P], bf16, tag="me")
                            nc.scalar.activation(
                                out=me[:], in_=mk_v[:, si * P:(si + 1) * P],
                                func=Act.Exp,
                                bias=sc_t[:, bi * TPB + t:bi * TPB + t + 1],
                                scale=1.0)
                            si += 1
                            lhs = me[:]
                        elif PATTERN[t] == "g":
                            j = (bi % SG) * NG + g_tiles.index(t)
                            lhs = me_g[:, j * P:(j + 1) * P]
                        else:
                            me = mep.tile([P, P], bf16, tag="me")
                            lcol = b * NV + v_tiles.index(t)
                            ecol = bi * NV + v_tiles.index(t)
                            nc.vector.tensor_scalar(
                                out=me[:], in0=iota_b[:],
                                scalar1=labf[:, lcol:lcol + 1],
                                scalar2=ev_t[:, ecol:ecol + 1],
                                op0=Alu.is_equal, op1=Alu.mult)
                            lhs = me[:]
                        nc.tensor.matmul(ps, lhsT=lhs,
                                         rhs=xw_v[:, t * W:(t + 1) * W],
                                         start=(t == 0), stop=(t == TPB - 1))
                    if bi % 2 == 1 or bi == nb - 1:
                        npsb = (bi % 2) + 1
                        lo = (bi - npsb + 1) * W
                        if EVICT == "dve":
                            nc.vector.tensor_copy(
                                ev[:, lo:lo + npsb * W],
                                ps2[:, :npsb * W])
                        else:
                            nc.scalar.copy(ev[:, lo:lo + npsb * W],
                                           ps2[:, :npsb * W])
                nc.sync.dma_start(out_d.ap()[ch, :, :nb * W],
                                  ev[:, :nb * W])

    nc.compile()
    return nc


# ---------------------------------------------------------------- host side
def _pack_blocks(counts):
    blocks = []
    s, nseg = 0, len(counts)
    while s < nseg:
        rows, s0 = 0, s
        while s < nseg and s - s0 < MAX_SEGS_PER_BLOCK:
            c = counts[s]
            if rows + c > ROWS_PER_BLOCK:
                break
            rows += int(c)
            s += 1
        assert s > s0, f"segment {s0} with {counts[s0]} rows exceeds a block"
        blocks.append((s0, s, rows))
    return blocks


def _numpy_fallback(x, labels, w, b):
    scores = x.astype(np.float64) @ w.astype(np.float64) + float(b)
    scores -= scores.max()
    e = np.exp(scores)
    a = e / e.sum()
    out = np.zeros((NUM_SEGMENTS, x.shape[1]), np.float64)
    np.add.at(out, labels, x * a[:, None])
    return out.astype(np.float32)


def kernel(x, monomer_labels_i, attn_w, attn_b):
    from concourse import bass_utils

    x = np.ascontiguousarray(np.asarray(x, dtype=np.float32))
    labels = np.asarray(monomer_labels_i).astype(np.int64)
    w = np.asarray(attn_w, dtype=np.float32)
    b = np.float32(np.asarray(attn_b))

    counts = np.bincount(labels, minlength=NUM_SEGMENTS)
    if np.abs(w).min() < 1e-30 or counts.max() > ROWS_PER_BLOCK:
        return _numpy_fallback(x, labels, w, b)

    order = np.argsort(labels, kind="stable")
    labels_s = labels[order]
    blocks = _pack_blocks(counts)
    nblocks = len(blocks)
    B = (nblocks + N_CORES - 1) // N_CORES
    NCHUNK = (B + CB - 1) // CB

    s0_arr = np.fromiter((blk[0] for blk in blocks), np.int64, nblocks)
    s1_arr = np.fromiter((blk[1] for blk in blocks), np.int64, nblocks)
    rows_arr = np.fromiter((blk[2] for blk in blocks), np.int64, nblocks)
    r0_arr = np.zeros(nblocks, np.int64)
    np.cumsum(rows_arr[:-1], out=r0_arr[1:])

    # per-row placement (all rows, sorted order)
    blk_r = np.repeat(np.arange(nblocks), rows_arr)
    pos = np.arange(N) - r0_arr[blk_r]
    tile_r = (pos >> 7).astype(np.int64)
    p_r = (pos & 127).astype(np.int64)
    lab_rel = (labels_s - s0_arr[blk_r]).astype(np.int64)
    core_r = blk_r // B
    bil_r = blk_r - core_r * B

    s_tiles = [t for t, c in enumerate(PATTERN) if c == "s"]
    g_tiles = [t for t, c in enumerate(PATTERN) if c == "g"]
    v_tiles = [t for t, c in enumerate(PATTERN) if c == "v"]
    s_slot = np.full(TPB, -1, np.int64)
    g_slot = np.full(TPB, -1, np.int64)
    v_slot = np.full(TPB, -1, np.int64)
    for j, t in enumerate(s_tiles):
        s_slot[t] = j
    for j, t in enumerate(g_tiles):
        g_slot[t] = j
    for j, t in enumerate(v_tiles):
        v_slot[t] = j

    xw = (x[order] * w[None, :]).astype(ml_dtypes.bfloat16)

    # combined xw_aug + mask bytes, [cores, B, P, BPB] u8
    xw_blk = np.zeros((N_CORES, B, P, TPB, W), ml_dtypes.bfloat16)
    xw_blk[:, :, :, :, 128] = 1.0
    xw_blk[core_r, bil_r, p_r, tile_r, 0:128] = xw

    byte_neg = np.array(-96.0, ml_dtypes.float8_e4m3).view(np.uint8)
    mk_blk = np.full((N_CORES, B, P, NS * P), byte_neg, np.uint8)
    m = s_slot[tile_r] >= 0
    mk_blk[core_r[m], bil_r[m], p_r[m],
           s_slot[tile_r[m]] * P + lab_rel[m]] = \
        np.array(0.0, ml_dtypes.float8_e4m3).view(np.uint8)

    xm_all = np.concatenate(
        [xw_blk.reshape(N_CORES, B, P, TPB * W).view(np.uint8)
         .reshape(N_CORES, B, P, TPB * W * 2),
         mk_blk], axis=3)                       # [cores, B, P, BPB]
    pad_blocks = NCHUNK * CB - B
    if pad_blocks:
        xm_all = np.concatenate(
            [xm_all, np.zeros((N_CORES, pad_blocks, P, BPB), np.uint8)],
            axis=1)
    xm_all = (xm_all.reshape(N_CORES, NCHUNK, CB, P, BPB)
              .transpose(0, 1, 3, 2, 4)
              .reshape(N_CORES, NCHUNK, P, CB * BPB))
    xm_all = np.ascontiguousarray(xm_all)

    labi_all = np.full((N_CORES, P, max(1, B * IDW)), -1, np.int16)
    if NG:
        m = g_slot[tile_r] >= 0
        labi_all[core_r[m], p_r[m],
                 bil_r[m] * IDW + g_slot[tile_r[m]]] = \
            (((bil_r[m] % SG) * NG + g_slot[tile_r[m]]) * P
             + lab_rel[m]).astype(np.int16)
    labf_all = np.full((N_CORES, P, max(1, B * NV)), 500.0, np.float32)
    if NV:
        m = v_slot[tile_r] >= 0
        labf_all[core_r[m], p_r[m],
                 bil_r[m] * NV + v_slot[tile_r[m]]] = lab_rel[m]

    in_maps = [{"xm": xm_all[c],
                "labi": labi_all[c],
                "labf": labf_all[c]} for c in range(N_CORES)]

    key = (B, PATTERN, SCORE, EVICT, RG, CB, SG)
    if key not in _COMPILED:
        _COMPILED[key] = _build_kernel(B)
    nc = _COMPILED[key]

    res = bass_utils.run_bass_kernel_spmd(nc, in_maps,
                                          core_ids=list(range(N_CORES)))

    # ---- gather / unshard
    Z = 0.0
    out = np.zeros((NUM_SEGMENTS, D), np.float32)
    for c in range(N_CORES):
        od = res.results[c]["out"].reshape(NCHUNK, P, CB, W)
        Z += float(od[:, :, :, 128].astype(np.float64).sum())
        od = od.transpose(0, 2, 1, 3)           # [NCHUNK, CB, P, W]
        for bi in range(B):
            gi = c * B + bi
            if gi >= nblocks:
                continue
            s0, s1 = int(s0_arr[gi]), int(s1_arr[gi])
            out[s0:s1] = od[bi // CB, bi % CB, :s1 - s0, :128]
    out /= (w[None, :].astype(np.float64) * Z)
    return out.astype(np.float32)


if __name__ == "__main__":
    from ref_io import get
    inputs, expected = get()
    out = kernel(**inputs)
    err = np.abs(out - expected)
    print("absmax err:", err.max(), "scale-rel:",
          err.max() / np.abs(expected).max())
